# revision 11
# baseline (speedup 1.0000x reference)
"""Block-sparse (BigBird-style) multi-head self-attention on 8 TRN2 NeuronCores.

Sharding: the Z = N*h = 32 (batch, head) pairs are split 4-per-core across 8
cores (head/data parallel).  QKV projections are column-split by head group, so
attention is fully local per core; the out_proj partial products are combined
with a ReduceScatter(add) inside each 4-core batch group, which also assigns
each core a distinct quarter of the output rows.
"""

import sys

sys.path.insert(0, "/opt/trn_rl_repo")

import numpy as np
import ml_dtypes

def _install_profile_hook():
    """Make NTFF profiling available under this container's axon setup.

    concourse.bass_utils reads the hook via ``antenv.axon_hooks``; the staged
    antenv lacks that module, so register an equivalent one backed by direct
    ctypes calls into libaxon_pjrt.so (same mechanism trn_agent_boot uses).
    Failure leaves a None hook -> tracing is skipped, runs still work.
    """
    import types

    try:
        import antenv.axon_hooks  # noqa: F401
        return
    except ImportError:
        pass
    hook = None
    try:
        from trn_agent_boot.trn_boot import _ntff_profile_via_ctypes
        hook = _ntff_profile_via_ctypes("/opt/axon/libaxon_pjrt.so")
    except Exception:
        hook = None
    try:
        import antenv
        mod = types.ModuleType("antenv.axon_hooks")
        mod._hook = hook
        mod.get_axon_ntff_profile_hook = lambda: mod._hook

        def _set(h):
            mod._hook = h

        mod.set_axon_ntff_profile_hook = _set
        sys.modules["antenv.axon_hooks"] = mod
        antenv.axon_hooks = mod
    except Exception:
        pass


_install_profile_hook()

import concourse.bass as bass
import concourse.bacc as bacc
import concourse.mybir as mybir
import concourse.tile as tile
from concourse.masks import make_identity
from concourse import bass_utils as _bass_utils
from concourse.bass_utils import run_bass_kernel_spmd

# The sandbox has no remote artifact store; keep profile artifacts local.
_bass_utils.upload_artifacts = lambda tmpdir: tmpdir

BF16 = mybir.dt.bfloat16
F32 = mybir.dt.float32
AFT = mybir.ActivationFunctionType

L, HID, HEADS, DH, B = 4096, 1024, 16, 64, 64
NB = L // B            # 64 key/query blocks
ZL = 4                 # z (batch*head) per core
NCORES = 8
GROUPS = [[0, 1, 2, 3], [4, 5, 6, 7]]
GSIZE = 4
NSUP = 8               # q supertiles of 512
RAND = 2


def _build(rand_idx, dbg=False):
    """Build + compile the SPMD program.  rand_idx: list of 62 (r0, r1) pairs."""
    nc = bacc.Bacc("TRN2", target_bir_lowering=False, debug=False,
                   num_devices=NCORES)
    dbg_t = {}
    if dbg:
        for name, shape, dt in (
            ("dqt", [128, L], BF16), ("dkt", [128, L], BF16),
            ("dvn", [128, 32 * 65], BF16), ("dvo", [128, 32 * 65], BF16),
            ("doth", [65, L], BF16), ("dot", [65, L], BF16),
            ("dpk", [128, L], BF16), ("dpp", [128, L], BF16),
            ("drec", [65, L], BF16),
        ):
            dbg_t[name] = nc.dram_tensor(name, shape, dt, kind="ExternalOutput")

    xt = nc.dram_tensor("xt", [HID, L], BF16, kind="ExternalInput")
    wqT = nc.dram_tensor("wqT", [HID, 256], BF16, kind="ExternalInput")
    wkT = nc.dram_tensor("wkT", [HID, 256], BF16, kind="ExternalInput")
    wvT = nc.dram_tensor("wvT", [HID, 256], BF16, kind="ExternalInput")
    woT = nc.dram_tensor("woT", [256, HID], BF16, kind="ExternalInput")
    qb = nc.dram_tensor("qb", [256], F32, kind="ExternalInput")
    kb = nc.dram_tensor("kb", [256], F32, kind="ExternalInput")
    vb = nc.dram_tensor("vb", [256], F32, kind="ExternalInput")
    ob = nc.dram_tensor("ob", [HID], F32, kind="ExternalInput")
    out = nc.dram_tensor("out", [256, L], F32, kind="ExternalOutput")

    from contextlib import ExitStack
    with tile.TileContext(nc) as tc:
        with ExitStack() as ctx:
            constp = ctx.enter_context(tc.tile_pool(name="const", bufs=1))
            wts = ctx.enter_context(tc.tile_pool(name="wts", bufs=1))
            acts = ctx.enter_context(tc.tile_pool(name="acts", bufs=1))
            vnat = ctx.enter_context(tc.tile_pool(name="vnat", bufs=1))
            otp = ctx.enter_context(tc.tile_pool(name="otp", bufs=1))
            vtpk = ctx.enter_context(tc.tile_pool(name="vtpk", bufs=2))
            xts = ctx.enter_context(tc.tile_pool(name="xts", bufs=9))
            ep = ctx.enter_context(tc.tile_pool(name="ep", bufs=3))
            ecp = ctx.enter_context(tc.tile_pool(name="ecp", bufs=2))
            stg = ctx.enter_context(tc.tile_pool(name="stg", bufs=2))
            ps_s = ctx.enter_context(tc.tile_pool(name="ps_s", bufs=2, space="PSUM"))
            ps_o = ctx.enter_context(tc.tile_pool(name="ps_o", bufs=2, space="PSUM"))
            ps_sa = ctx.enter_context(tc.tile_pool(name="ps_sa", bufs=2, space="PSUM"))
            ps_pk = ctx.enter_context(tc.tile_pool(name="ps_pk", bufs=2, space="PSUM"))
            dram = ctx.enter_context(tc.tile_pool(name="dram", bufs=1, space="DRAM"))
            # ---------------- constants ----------------
            ident = constp.tile([128, 64], BF16, tag="ident")
            make_identity(nc, ident[0:64, :])
            make_identity(nc, ident[64:128, :])
            ones64 = constp.tile([64, 64], BF16, tag="ones64")
            nc.gpsimd.memset(ones64[:], 1.0)
            ones128 = constp.tile([128, 64], BF16, tag="ones128")
            nc.gpsimd.memset(ones128[:], 1.0)
            zaug = constp.tile([64, 65], BF16, tag="zaug")
            nc.gpsimd.memset(zaug[:], 0.0)
            nc.gpsimd.memset(zaug[:, 64:65], 1.0)

            # ---------------- weights / biases ----------------
            w_sb = {}
            for name, t in (("q", wqT), ("k", wkT), ("v", wvT)):
                w = wts.tile([128, 8 * 256], BF16, tag=f"w{name}")
                for k in range(8):
                    nc.sync.dma_start(w[:, k * 256:(k + 1) * 256],
                                      t[k * 128:(k + 1) * 128, :])
                w_sb[name] = w
            wo_sb = []
            for cc in range(2):
                w = wts.tile([128, HID], BF16, tag=f"wo{cc}")
                nc.sync.dma_start(w[:], woT[cc * 128:(cc + 1) * 128, :])
                wo_sb.append(w)
            b_sb = {}
            for name, t in (("q", qb), ("k", kb), ("v", vb)):
                bt = wts.tile([128, 2], F32, tag=f"b{name}")
                nc.sync.dma_start(bt[:], t.ap().rearrange("(c p) -> p c", p=128))
                b_sb[name] = bt
            ob_sb = wts.tile([128, 8], F32, tag="ob")
            nc.sync.dma_start(ob_sb[:], ob.ap().rearrange("(m p) -> p m", p=128))

            # ---------------- phase 1: QKV projections ----------------
            # QT/KT/VT[cc][c_local 128, l 4096] (bf16), c_local = z-pair rows
            QT = [acts.tile([128, L], BF16, tag=f"qt{i}", name=f"QT{i}") for i in range(2)]
            KT = [acts.tile([128, L], BF16, tag=f"kt{i}", name=f"KT{i}") for i in range(2)]
            VT = [vtpk.tile([128, L], BF16, tag="vtpk", name=f"VT{i}") for i in range(2)]
            dests = {"q": QT, "k": KT, "v": VT}

            for ls in range(NSUP):
                xtile = [xts.tile([128, 512], BF16, tag="xt", name=f"xt_{ls}_{k}") for k in range(8)]
                for k in range(8):
                    nc.sync.dma_start(
                        xtile[k][:],
                        xt[k * 128:(k + 1) * 128, ls * 512:(ls + 1) * 512])
                for pname in ("q", "k", "v"):
                    for cc in range(2):
                        ps = ps_s.tile([128, 512], F32, tag="s")
                        for k in range(8):
                            nc.tensor.matmul(
                                ps[:],
                                w_sb[pname][:, k * 256 + cc * 128:
                                            k * 256 + (cc + 1) * 128],
                                xtile[k][:],
                                start=(k == 0), stop=(k == 7))
                        nc.scalar.activation(
                            dests[pname][cc][:, ls * 512:(ls + 1) * 512],
                            ps[:], AFT.Identity,
                            bias=b_sb[pname][:, cc:cc + 1])

            if dbg:
                nc.sync.dma_start(dbg_t["dqt"].ap(), QT[0][:])
                nc.sync.dma_start(dbg_t["dkt"].ap(), KT[0][:])

            # ---------------- phase 2: V -> natural layout ----------------
            # Vn[z] chunk ch = key blocks (2ch, 2ch+1) [128, 65] (col 64 = ones)
            # Vo[z] chunk ch = key blocks (2ch+1, 2ch+2)
            Vn = [vnat.tile([128, 32 * 65], BF16, tag=f"vn{z}", name=f"Vn{z}") for z in range(ZL)]
            Vo = [vnat.tile([128, 32 * 65], BF16, tag=f"vo{z}", name=f"Vo{z}") for z in range(ZL)]
            for z in range(ZL):
                zp, base = z // 2, 64 * (z % 2)
                vsl = VT[zp][base:base + 64, :]
                idn = ident[base:base + 64, :]
                for ch in range(32):
                    pt = ps_sa.tile([128, 64], BF16, tag="sa")
                    nc.tensor.transpose(
                        pt[:], vsl[:, ch * 128:(ch + 1) * 128], idn)
                    nc.vector.tensor_copy(
                        Vn[z][:, ch * 65:ch * 65 + 64], pt[:])
                for ch in range(32):
                    c0 = 64 + ch * 128
                    w = min(128, L - c0)
                    pt = ps_sa.tile([128, 64], BF16, tag="sa")
                    nc.tensor.transpose(
                        pt[0:w, :], vsl[:, c0:c0 + w], idn)
                    nc.vector.tensor_copy(
                        Vo[z][0:w, ch * 65:ch * 65 + 64], pt[0:w, :])
                nc.gpsimd.memset(
                    Vn[z][:].rearrange("p (c w) -> p c w", w=65)[:, :, 64:65], 1.0)
                nc.gpsimd.memset(
                    Vo[z][:].rearrange("p (c w) -> p c w", w=65)[:, :, 64:65], 1.0)

            def v_single(z, blk):
                """[64, 65] value block (keys on partitions 0:64), ones col."""
                if blk % 2 == 0:
                    return Vn[z][0:64, (blk // 2) * 65:(blk // 2) * 65 + 65]
                return Vo[z][0:64, ((blk - 1) // 2) * 65:((blk - 1) // 2) * 65 + 65]

            def v_window(z, blk):
                """[128, 65] for key blocks (blk, blk+1)."""
                if blk % 2 == 0:
                    return Vn[z][:, (blk // 2) * 65:(blk // 2) * 65 + 65]
                return Vo[z][:, ((blk - 1) // 2) * 65:((blk - 1) // 2) * 65 + 65]

            if dbg:
                nc.sync.dma_start(dbg_t["dvn"].ap(), Vn[0][:])
                nc.sync.dma_start(dbg_t["dvo"].ap(), Vo[0][:])

            # ---------------- phase 3: attention per z ----------------
            OT = [otp.tile([65, L], BF16, tag=f"ot{z}", name=f"OT{z}") for z in range(ZL)]
            for z in range(ZL):
                zp, base = z // 2, 64 * (z % 2)
                KTz = KT[zp][base:base + 64, :]
                QTz = QT[zp][base:base + 64, :]

                # --- head rows: q blocks 0,1 attend to all keys ---
                po = ps_o.tile([65, 128], F32, tag="o")
                for ch in range(32):
                    ss = ps_s.tile([128, 128], F32, tag="s")
                    nc.tensor.matmul(ss[:], KTz[:, ch * 128:(ch + 1) * 128],
                                     QTz[:, 0:128], start=True, stop=True)
                    eh = ep.tile([128, 128], BF16, tag="eh")
                    nc.scalar.activation(eh[:], ss[:], AFT.Exp)
                    nc.tensor.matmul(po[:], Vn[z][:, ch * 65:ch * 65 + 65],
                                     eh[:], start=(ch == 0), stop=(ch == 31))
                nc.scalar.copy(OT[z][:, 0:128], po[:])

                # --- middle rows: blocks 2..63 ---
                for sup in range(NSUP):
                    b0 = max(2, 8 * sup)
                    b1 = 8 * (sup + 1)
                    q0, qw = b0 * 64, (b1 - b0) * 64
                    sc = ps_s.tile([128, 512], F32, tag="s")
                    nc.tensor.matmul(sc[:, 0:qw], KTz[:, 0:128],
                                     QTz[:, q0:q0 + qw], start=True, stop=True)
                    ec = ecp.tile([128, 512], BF16, tag="ec")
                    nc.scalar.activation(ec[:, 0:qw], sc[:, 0:qw], AFT.Exp)

                    for i in range(b0, b1):
                        qs = slice((i - b0) * 64, (i - b0) * 64 + 64)
                        qq = QTz[:, i * 64:(i + 1) * 64]
                        r0, r1 = rand_idx[i - 2]
                        po = ps_o.tile([65, 64], F32, tag="o")
                        if i < 63:
                            # scores: sliding window (i-1, i) | packed (i+1, r0, r1)
                            sa = ps_sa.tile([128, 64], F32, tag="sa")
                            nc.tensor.matmul(
                                sa[:], KTz[:, (i - 1) * 64:(i + 1) * 64], qq,
                                start=True, stop=True)
                            esa = ep.tile([128, 64], BF16, tag="esa")
                            nc.scalar.activation(esa[:], sa[:], AFT.Exp)
                            pk = ps_pk.tile([64, 192], F32, tag="pk")
                            for j, blk in enumerate((i + 1, r0, r1)):
                                nc.tensor.matmul(
                                    pk[:, j * 64:(j + 1) * 64],
                                    KTz[:, blk * 64:(blk + 1) * 64], qq,
                                    start=True, stop=True)
                            epk = ep.tile([64, 192], BF16, tag="epk")
                            nc.scalar.activation(epk[:], pk[:], AFT.Exp)

                            nc.tensor.matmul(po[:], Vn[z][:, 0:65],
                                             ec[:, qs], start=True, stop=False)
                            nc.tensor.matmul(po[:], v_window(z, i - 1),
                                             esa[:], start=False, stop=False)
                            nc.tensor.matmul(po[:], v_single(z, i + 1),
                                             epk[:, 0:64], start=False, stop=False)
                            nc.tensor.matmul(po[:], v_single(z, r0),
                                             epk[:, 64:128], start=False, stop=False)
                            nc.tensor.matmul(po[:], v_single(z, r1),
                                             epk[:, 128:192], start=False, stop=True)
                        else:
                            # last block row: key layout [ZERO, k62, k63],
                            # value layout [v62, v63, ZERO]
                            pk = ps_pk.tile([64, 256], F32, tag="pk")
                            for j, blk in enumerate((62, 63, r0, r1)):
                                nc.tensor.matmul(
                                    pk[:, j * 64:(j + 1) * 64],
                                    KTz[:, blk * 64:(blk + 1) * 64], qq,
                                    start=True, stop=True)
                            epk = ep.tile([64, 256], BF16, tag="epk")
                            nc.scalar.activation(epk[:], pk[:], AFT.Exp)

                            nc.tensor.matmul(po[:], Vn[z][:, 0:65],
                                             ec[:, qs], start=True, stop=False)
                            # exp(0)=1 scores vs ZERO key block pair with v62
                            nc.tensor.matmul(po[:], v_single(z, 62),
                                             ones64[:], start=False, stop=False)
                            # scores vs k62 pair with v63
                            nc.tensor.matmul(po[:], v_single(z, 63),
                                             epk[:, 0:64], start=False, stop=False)
                            # scores vs k63 pair with ZERO values (denominator only)
                            nc.tensor.matmul(po[:], zaug[:],
                                             epk[:, 64:128], start=False, stop=False)
                            nc.tensor.matmul(po[:], v_single(z, r0),
                                             epk[:, 128:192], start=False, stop=False)
                            nc.tensor.matmul(po[:], v_single(z, r1),
                                             epk[:, 192:256], start=False, stop=True)
                        nc.scalar.copy(OT[z][:, i * 64:(i + 1) * 64], po[:])

                # --- softmax normalization: rows 0:64 /= row 64 ---
                if dbg and z == 0:
                    nc.sync.dma_start(dbg_t["doth"].ap(), OT[0][:])
                with nc.allow_low_precision("softmax denominator recip in bf16"):
                    nc.vector.reciprocal(OT[z][64:65, :], OT[z][64:65, :])
                if dbg and z == 0:
                    nc.sync.dma_start(dbg_t["drec"].ap(), OT[0][:])
                # broadcast the reciprocal row across 64 partitions via a
                # K=1 matmul with a ones column (partition_broadcast is
                # unreliable on HW), then scale in place.
                for ls in range(NSUP):
                    rb = ps_pk.tile([64, 512], F32, tag="pk")
                    nc.tensor.matmul(rb[:], ones128[64:65, 0:64],
                                     OT[z][64:65, ls * 512:(ls + 1) * 512],
                                     start=True, stop=True)
                    nc.vector.tensor_mul(OT[z][0:64, ls * 512:(ls + 1) * 512],
                                         OT[z][0:64, ls * 512:(ls + 1) * 512],
                                         rb[:])

            if dbg:
                nc.sync.dma_start(dbg_t["dot"].ap(), OT[0][:])

            # ---------------- phase 4: pack z-pairs [128, L] ----------------
            PK = [vtpk.tile([128, L], BF16, tag="vtpk", name=f"PK{i}") for i in range(2)]
            for zp in range(2):
                nc.sync.dma_start(PK[zp][0:64, :], OT[2 * zp][0:64, :])
                nc.sync.dma_start(PK[zp][64:128, :], OT[2 * zp + 1][0:64, :])

            if dbg:
                nc.sync.dma_start(dbg_t["dpk"].ap(), PK[0][:])

            # ---------------- phase 5: out_proj partial ----------------
            partial = dram.tile([HID, L], BF16)
            for mc in range(8):
                for lt in range(8):
                    ps = ps_s.tile([128, 512], F32, tag="s")
                    nc.tensor.matmul(ps[:],
                                     wo_sb[0][:, mc * 128:(mc + 1) * 128],
                                     PK[0][:, lt * 512:(lt + 1) * 512],
                                     start=True, stop=False)
                    nc.tensor.matmul(ps[:],
                                     wo_sb[1][:, mc * 128:(mc + 1) * 128],
                                     PK[1][:, lt * 512:(lt + 1) * 512],
                                     start=False, stop=True)
                    st = stg.tile([128, 512], BF16, tag="st")
                    nc.scalar.activation(st[:], ps[:], AFT.Identity,
                                         bias=ob_sb[:, mc:mc + 1])
                    nc.sync.dma_start(
                        partial[mc * 128:(mc + 1) * 128, lt * 512:(lt + 1) * 512],
                        st[:])

            if dbg:
                nc.sync.dma_start(dbg_t["dpp"].ap(), partial[0:128, :])

            # ---------------- phase 6: ReduceScatter + emit ----------------
            rs_out = dram.tile([256, L], BF16)
            nc.gpsimd.collective_compute(
                "ReduceScatter", mybir.AluOpType.add, replica_groups=GROUPS,
                ins=[partial.opt()], outs=[rs_out.opt()])
            for rc in range(2):
                for lt in range(8):
                    bt = stg.tile([128, 512], BF16, tag="fin_b")
                    nc.sync.dma_start(
                        bt[:],
                        rs_out[rc * 128:(rc + 1) * 128, lt * 512:(lt + 1) * 512])
                    ft = stg.tile([128, 512], F32, tag="fin_f")
                    nc.vector.tensor_copy(ft[:], bt[:])
                    nc.sync.dma_start(
                        out[rc * 128:(rc + 1) * 128, lt * 512:(lt + 1) * 512],
                        ft[:])

    nc.compile()
    return nc


_CACHE = {}


def _get_nc(rand_idx_key, rand_idx):
    if rand_idx_key not in _CACHE:
        _CACHE[rand_idx_key] = _build(rand_idx)
    return _CACHE[rand_idx_key]


def make_in_maps(query, q_proj, q_bias, k_proj, k_bias, v_proj, v_bias,
                 out_proj, out_bias, rand_indices, h, block_size):
    bf = ml_dtypes.bfloat16
    scale = DH ** -0.5
    in_maps = []
    for c in range(NCORES):
        n, g = c // 4, c % 4
        cols = slice(g * 256, (g + 1) * 256)
        m = {
            "xt": np.ascontiguousarray(query[:, n, :].T).astype(bf),
            "wqT": np.ascontiguousarray((q_proj[cols, :] * scale).T).astype(bf),
            "wkT": np.ascontiguousarray(k_proj[cols, :].T).astype(bf),
            "wvT": np.ascontiguousarray(v_proj[cols, :].T).astype(bf),
            "woT": np.ascontiguousarray(out_proj[:, cols].T).astype(bf),
            "qb": np.ascontiguousarray(q_bias[cols] * scale).astype(np.float32),
            "kb": np.ascontiguousarray(k_bias[cols]).astype(np.float32),
            "vb": np.ascontiguousarray(v_bias[cols]).astype(np.float32),
            "ob": np.ascontiguousarray(out_bias * 0.25).astype(np.float32),
        }
        in_maps.append(m)
    return in_maps


def kernel(query, q_proj, q_bias, k_proj, k_bias, v_proj, v_bias,
           out_proj, out_bias, rand_indices, h, block_size, _results_hook=None):
    query = np.asarray(query, dtype=np.float32)
    q_proj = np.asarray(q_proj, dtype=np.float32)
    q_bias = np.asarray(q_bias, dtype=np.float32)
    k_proj = np.asarray(k_proj, dtype=np.float32)
    k_bias = np.asarray(k_bias, dtype=np.float32)
    v_proj = np.asarray(v_proj, dtype=np.float32)
    v_bias = np.asarray(v_bias, dtype=np.float32)
    out_proj = np.asarray(out_proj, dtype=np.float32)
    out_bias = np.asarray(out_bias, dtype=np.float32)
    rand_indices = np.asarray(rand_indices)
    assert int(h) == HEADS and int(block_size) == B
    assert query.shape == (L, 2, HID)
    assert rand_indices.shape == (NB - 2, RAND)
    rand_idx = [(int(r0), int(r1)) for r0, r1 in rand_indices]
    assert all(2 <= r < NB for p in rand_idx for r in p)

    nc = _get_nc(tuple(map(tuple, rand_idx)), rand_idx)
    in_maps = make_in_maps(query, q_proj, q_bias, k_proj, k_bias, v_proj,
                           v_bias, out_proj, out_bias, rand_indices, h,
                           block_size)
    res = run_bass_kernel_spmd(nc, in_maps, core_ids=list(range(NCORES)))
    if _results_hook is not None:
        _results_hook(res)

    output = np.empty((L, 2, HID), dtype=np.float32)
    for c in range(NCORES):
        n, g = c // 4, c % 4
        output[:, n, g * 256:(g + 1) * 256] = res.results[c]["out"].T
    return output


# revision 13
# speedup vs baseline: 1.0430x; 1.0430x over previous
"""Block-sparse (BigBird-style) multi-head self-attention on 8 TRN2 NeuronCores.

Sharding: the Z = N*h = 32 (batch, head) pairs are split 4-per-core across 8
cores (head/data parallel).  QKV projections are column-split by head group, so
attention is fully local per core; the out_proj partial products are combined
with a ReduceScatter(add) inside each 4-core batch group, which also assigns
each core a distinct quarter of the output rows.
"""

import sys

sys.path.insert(0, "/opt/trn_rl_repo")

import numpy as np
import ml_dtypes

def _install_profile_hook():
    """Make NTFF profiling available under this container's axon setup.

    concourse.bass_utils reads the hook via ``antenv.axon_hooks``; the staged
    antenv lacks that module, so register an equivalent one backed by direct
    ctypes calls into libaxon_pjrt.so (same mechanism trn_agent_boot uses).
    Failure leaves a None hook -> tracing is skipped, runs still work.
    """
    import types

    try:
        import antenv.axon_hooks  # noqa: F401
        return
    except ImportError:
        pass
    hook = None
    try:
        from trn_agent_boot.trn_boot import _ntff_profile_via_ctypes
        hook = _ntff_profile_via_ctypes("/opt/axon/libaxon_pjrt.so")
    except Exception:
        hook = None
    try:
        import antenv
        mod = types.ModuleType("antenv.axon_hooks")
        mod._hook = hook
        mod.get_axon_ntff_profile_hook = lambda: mod._hook

        def _set(h):
            mod._hook = h

        mod.set_axon_ntff_profile_hook = _set
        sys.modules["antenv.axon_hooks"] = mod
        antenv.axon_hooks = mod
    except Exception:
        pass


_install_profile_hook()

import concourse.bass as bass
import concourse.bacc as bacc
import concourse.mybir as mybir
import concourse.tile as tile
from concourse.masks import make_identity
from concourse import bass_utils as _bass_utils
from concourse.bass_utils import run_bass_kernel_spmd

# The sandbox has no remote artifact store; keep profile artifacts local.
_bass_utils.upload_artifacts = lambda tmpdir: tmpdir

BF16 = mybir.dt.bfloat16
F32 = mybir.dt.float32
AFT = mybir.ActivationFunctionType

L, HID, HEADS, DH, B = 4096, 1024, 16, 64, 64
NB = L // B            # 64 key/query blocks
ZL = 4                 # z (batch*head) per core
NCORES = 8
GROUPS = [[0, 1, 2, 3], [4, 5, 6, 7]]
GSIZE = 4
NSUP = 8               # q supertiles of 512
RAND = 2


def _build(rand_idx, dbg=False):
    """Build + compile the SPMD program.  rand_idx: list of 62 (r0, r1) pairs."""
    nc = bacc.Bacc("TRN2", target_bir_lowering=False, debug=False,
                   num_devices=NCORES)
    dbg_t = {}
    if dbg:
        for name, shape, dt in (
            ("dqt", [128, L], BF16), ("dkt", [128, L], BF16),
            ("dvn", [128, 32 * 65], BF16), ("dvo", [128, 32 * 65], BF16),
            ("doth", [65, L], BF16), ("dot", [65, L], BF16),
            ("dpk", [128, L], BF16), ("dpp", [128, L], BF16),
            ("drec", [65, L], BF16),
        ):
            dbg_t[name] = nc.dram_tensor(name, shape, dt, kind="ExternalOutput")

    xt = nc.dram_tensor("xt", [HID, L], BF16, kind="ExternalInput")
    wqT = nc.dram_tensor("wqT", [HID, 256], BF16, kind="ExternalInput")
    wkT = nc.dram_tensor("wkT", [HID, 256], BF16, kind="ExternalInput")
    wvT = nc.dram_tensor("wvT", [HID, 256], BF16, kind="ExternalInput")
    woT = nc.dram_tensor("woT", [256, HID], BF16, kind="ExternalInput")
    qb = nc.dram_tensor("qb", [256], F32, kind="ExternalInput")
    kb = nc.dram_tensor("kb", [256], F32, kind="ExternalInput")
    vb = nc.dram_tensor("vb", [256], F32, kind="ExternalInput")
    ob = nc.dram_tensor("ob", [HID], F32, kind="ExternalInput")
    out = nc.dram_tensor("out", [256, L], F32, kind="ExternalOutput")

    from contextlib import ExitStack
    with tile.TileContext(nc) as tc:
        with ExitStack() as ctx:
            constp = ctx.enter_context(tc.tile_pool(name="const", bufs=1))
            wts = ctx.enter_context(tc.tile_pool(name="wts", bufs=1))
            acts = ctx.enter_context(tc.tile_pool(name="acts", bufs=1))
            vnat = ctx.enter_context(tc.tile_pool(name="vnat", bufs=1))
            otp = ctx.enter_context(tc.tile_pool(name="otp", bufs=1))
            vtpk = ctx.enter_context(tc.tile_pool(name="vtpk", bufs=2))
            xts = ctx.enter_context(tc.tile_pool(name="xts", bufs=9))
            ep = ctx.enter_context(tc.tile_pool(name="ep", bufs=3))
            ecp = ctx.enter_context(tc.tile_pool(name="ecp", bufs=2))
            stg = ctx.enter_context(tc.tile_pool(name="stg", bufs=2))
            ps_s = ctx.enter_context(tc.tile_pool(name="ps_s", bufs=2, space="PSUM"))
            ps_o = ctx.enter_context(tc.tile_pool(name="ps_o", bufs=2, space="PSUM"))
            ps_sa = ctx.enter_context(tc.tile_pool(name="ps_sa", bufs=2, space="PSUM"))
            ps_pk = ctx.enter_context(tc.tile_pool(name="ps_pk", bufs=2, space="PSUM"))
            dram = ctx.enter_context(tc.tile_pool(name="dram", bufs=1, space="DRAM"))
            # ---------------- constants ----------------
            ident = constp.tile([128, 64], BF16, tag="ident")
            make_identity(nc, ident[0:64, :])
            make_identity(nc, ident[64:128, :])
            ones64 = constp.tile([64, 64], BF16, tag="ones64")
            nc.gpsimd.memset(ones64[:], 1.0)
            ones128 = constp.tile([128, 64], BF16, tag="ones128")
            nc.gpsimd.memset(ones128[:], 1.0)
            zaug = constp.tile([64, 65], BF16, tag="zaug")
            nc.gpsimd.memset(zaug[:], 0.0)
            nc.gpsimd.memset(zaug[:, 64:65], 1.0)

            # ---------------- weights / biases ----------------
            w_sb = {}
            for name, t in (("q", wqT), ("k", wkT), ("v", wvT)):
                w = wts.tile([128, 8 * 256], BF16, tag=f"w{name}")
                for k in range(8):
                    nc.sync.dma_start(w[:, k * 256:(k + 1) * 256],
                                      t[k * 128:(k + 1) * 128, :])
                w_sb[name] = w
            wo_sb = []
            for cc in range(2):
                w = wts.tile([128, HID], BF16, tag=f"wo{cc}")
                nc.sync.dma_start(w[:], woT[cc * 128:(cc + 1) * 128, :])
                wo_sb.append(w)
            b_sb = {}
            for name, t in (("q", qb), ("k", kb), ("v", vb)):
                bt = wts.tile([128, 2], F32, tag=f"b{name}")
                nc.sync.dma_start(bt[:], t.ap().rearrange("(c p) -> p c", p=128))
                b_sb[name] = bt
            ob_sb = wts.tile([128, 8], F32, tag="ob")
            nc.sync.dma_start(ob_sb[:], ob.ap().rearrange("(m p) -> p m", p=128))

            # ---------------- phase 1: QKV projections ----------------
            # QT/KT/VT[cc][c_local 128, l 4096] (bf16), c_local = z-pair rows
            QT = [acts.tile([128, L], BF16, tag=f"qt{i}", name=f"QT{i}") for i in range(2)]
            KT = [acts.tile([128, L], BF16, tag=f"kt{i}", name=f"KT{i}") for i in range(2)]
            VT = [vtpk.tile([128, L], BF16, tag="vtpk", name=f"VT{i}") for i in range(2)]
            dests = {"q": QT, "k": KT, "v": VT}

            for ls in range(NSUP):
                xtile = [xts.tile([128, 512], BF16, tag="xt", name=f"xt_{ls}_{k}") for k in range(8)]
                for k in range(8):
                    nc.sync.dma_start(
                        xtile[k][:],
                        xt[k * 128:(k + 1) * 128, ls * 512:(ls + 1) * 512])
                for pname in ("q", "k", "v"):
                    for cc in range(2):
                        ps = ps_s.tile([128, 512], F32, tag="s")
                        for k in range(8):
                            nc.tensor.matmul(
                                ps[:],
                                w_sb[pname][:, k * 256 + cc * 128:
                                            k * 256 + (cc + 1) * 128],
                                xtile[k][:],
                                start=(k == 0), stop=(k == 7))
                        nc.vector.tensor_scalar_add(
                            dests[pname][cc][:, ls * 512:(ls + 1) * 512],
                            ps[:], b_sb[pname][:, cc:cc + 1])

            if dbg:
                nc.sync.dma_start(dbg_t["dqt"].ap(), QT[0][:])
                nc.sync.dma_start(dbg_t["dkt"].ap(), KT[0][:])

            # ---------------- phase 2: V -> natural layout ----------------
            # Vn[z] chunk ch = key blocks (2ch, 2ch+1) [128, 65] (col 64 = ones)
            # Vo[z] chunk ch = key blocks (2ch+1, 2ch+2)
            Vn = [vnat.tile([128, 32 * 65], BF16, tag=f"vn{z}", name=f"Vn{z}") for z in range(ZL)]
            Vo = [vnat.tile([128, 32 * 65], BF16, tag=f"vo{z}", name=f"Vo{z}") for z in range(ZL)]
            for z in range(ZL):
                zp, base = z // 2, 64 * (z % 2)
                vsl = VT[zp][base:base + 64, :]
                idn = ident[base:base + 64, :]
                for ch in range(32):
                    pt = ps_sa.tile([128, 64], BF16, tag="sa")
                    nc.tensor.transpose(
                        pt[:], vsl[:, ch * 128:(ch + 1) * 128], idn)
                    nc.vector.tensor_copy(
                        Vn[z][:, ch * 65:ch * 65 + 64], pt[:])
                for ch in range(32):
                    c0 = 64 + ch * 128
                    w = min(128, L - c0)
                    pt = ps_sa.tile([128, 64], BF16, tag="sa")
                    nc.tensor.transpose(
                        pt[0:w, :], vsl[:, c0:c0 + w], idn)
                    nc.vector.tensor_copy(
                        Vo[z][0:w, ch * 65:ch * 65 + 64], pt[0:w, :])
                nc.gpsimd.memset(
                    Vn[z][:].rearrange("p (c w) -> p c w", w=65)[:, :, 64:65], 1.0)
                nc.gpsimd.memset(
                    Vo[z][:].rearrange("p (c w) -> p c w", w=65)[:, :, 64:65], 1.0)

            def v_single(z, blk):
                """[64, 65] value block (keys on partitions 0:64), ones col."""
                if blk % 2 == 0:
                    return Vn[z][0:64, (blk // 2) * 65:(blk // 2) * 65 + 65]
                return Vo[z][0:64, ((blk - 1) // 2) * 65:((blk - 1) // 2) * 65 + 65]

            def v_window(z, blk):
                """[128, 65] for key blocks (blk, blk+1)."""
                if blk % 2 == 0:
                    return Vn[z][:, (blk // 2) * 65:(blk // 2) * 65 + 65]
                return Vo[z][:, ((blk - 1) // 2) * 65:((blk - 1) // 2) * 65 + 65]

            if dbg:
                nc.sync.dma_start(dbg_t["dvn"].ap(), Vn[0][:])
                nc.sync.dma_start(dbg_t["dvo"].ap(), Vo[0][:])

            # ---------------- phase 3: attention per z ----------------
            OT = [otp.tile([65, L], BF16, tag=f"ot{z}", name=f"OT{z}") for z in range(ZL)]
            for z in range(ZL):
                zp, base = z // 2, 64 * (z % 2)
                KTz = KT[zp][base:base + 64, :]
                QTz = QT[zp][base:base + 64, :]

                # --- head rows: q blocks 0,1 attend to all keys ---
                po = ps_o.tile([65, 128], F32, tag="o")
                for ch in range(32):
                    ss = ps_s.tile([128, 128], F32, tag="s")
                    nc.tensor.matmul(ss[:], KTz[:, ch * 128:(ch + 1) * 128],
                                     QTz[:, 0:128], start=True, stop=True)
                    eh = ep.tile([128, 128], BF16, tag="eh")
                    nc.scalar.activation(eh[:], ss[:], AFT.Exp)
                    nc.tensor.matmul(po[:], Vn[z][:, ch * 65:ch * 65 + 65],
                                     eh[:], start=(ch == 0), stop=(ch == 31))
                nc.vector.tensor_copy(OT[z][:, 0:128], po[:])

                # --- middle rows: blocks 2..63 ---
                for sup in range(NSUP):
                    b0 = max(2, 8 * sup)
                    b1 = 8 * (sup + 1)
                    q0, qw = b0 * 64, (b1 - b0) * 64
                    sc = ps_s.tile([128, 512], F32, tag="s")
                    nc.tensor.matmul(sc[:, 0:qw], KTz[:, 0:128],
                                     QTz[:, q0:q0 + qw], start=True, stop=True)
                    ec = ecp.tile([128, 512], BF16, tag="ec")
                    nc.scalar.activation(ec[:, 0:qw], sc[:, 0:qw], AFT.Exp)

                    for i in range(b0, b1):
                        qs = slice((i - b0) * 64, (i - b0) * 64 + 64)
                        qq = QTz[:, i * 64:(i + 1) * 64]
                        r0, r1 = rand_idx[i - 2]
                        po = ps_o.tile([65, 64], F32, tag="o")
                        if i < 63:
                            # scores: sliding window (i-1, i) | packed (i+1, r0, r1)
                            sa = ps_sa.tile([128, 64], F32, tag="sa")
                            nc.tensor.matmul(
                                sa[:], KTz[:, (i - 1) * 64:(i + 1) * 64], qq,
                                start=True, stop=True)
                            esa = ep.tile([128, 64], BF16, tag="esa")
                            nc.scalar.activation(esa[:], sa[:], AFT.Exp)
                            pk = ps_pk.tile([64, 192], F32, tag="pk")
                            for j, blk in enumerate((i + 1, r0, r1)):
                                nc.tensor.matmul(
                                    pk[:, j * 64:(j + 1) * 64],
                                    KTz[:, blk * 64:(blk + 1) * 64], qq,
                                    start=True, stop=True)
                            epk = ep.tile([64, 192], BF16, tag="epk")
                            nc.scalar.activation(epk[:], pk[:], AFT.Exp)

                            nc.tensor.matmul(po[:], Vn[z][:, 0:65],
                                             ec[:, qs], start=True, stop=False)
                            nc.tensor.matmul(po[:], v_window(z, i - 1),
                                             esa[:], start=False, stop=False)
                            nc.tensor.matmul(po[:], v_single(z, i + 1),
                                             epk[:, 0:64], start=False, stop=False)
                            nc.tensor.matmul(po[:], v_single(z, r0),
                                             epk[:, 64:128], start=False, stop=False)
                            nc.tensor.matmul(po[:], v_single(z, r1),
                                             epk[:, 128:192], start=False, stop=True)
                        else:
                            # last block row: key layout [ZERO, k62, k63],
                            # value layout [v62, v63, ZERO]
                            pk = ps_pk.tile([64, 256], F32, tag="pk")
                            for j, blk in enumerate((62, 63, r0, r1)):
                                nc.tensor.matmul(
                                    pk[:, j * 64:(j + 1) * 64],
                                    KTz[:, blk * 64:(blk + 1) * 64], qq,
                                    start=True, stop=True)
                            epk = ep.tile([64, 256], BF16, tag="epk")
                            nc.scalar.activation(epk[:], pk[:], AFT.Exp)

                            nc.tensor.matmul(po[:], Vn[z][:, 0:65],
                                             ec[:, qs], start=True, stop=False)
                            # exp(0)=1 scores vs ZERO key block pair with v62
                            nc.tensor.matmul(po[:], v_single(z, 62),
                                             ones64[:], start=False, stop=False)
                            # scores vs k62 pair with v63
                            nc.tensor.matmul(po[:], v_single(z, 63),
                                             epk[:, 0:64], start=False, stop=False)
                            # scores vs k63 pair with ZERO values (denominator only)
                            nc.tensor.matmul(po[:], zaug[:],
                                             epk[:, 64:128], start=False, stop=False)
                            nc.tensor.matmul(po[:], v_single(z, r0),
                                             epk[:, 128:192], start=False, stop=False)
                            nc.tensor.matmul(po[:], v_single(z, r1),
                                             epk[:, 192:256], start=False, stop=True)
                        nc.vector.tensor_copy(OT[z][:, i * 64:(i + 1) * 64], po[:])

                # --- softmax normalization: rows 0:64 /= row 64 ---
                # Broadcast D across 64 partitions via a K=1 matmul with a
                # ones column (partition_broadcast is unreliable on HW, and
                # a 1-partition reciprocal is ~25us), then wide recip + mul.
                if dbg and z == 0:
                    nc.sync.dma_start(dbg_t["doth"].ap(), OT[0][:])
                for ls in range(NSUP):
                    rb = ps_pk.tile([64, 512], F32, tag="pk")
                    nc.tensor.matmul(rb[:], ones128[64:65, 0:64],
                                     OT[z][64:65, ls * 512:(ls + 1) * 512],
                                     start=True, stop=True)
                    rr = ep.tile([64, 512], BF16, tag="rr")
                    with nc.allow_low_precision("softmax denom recip bf16"):
                        nc.vector.reciprocal(rr[:], rb[:])
                    nc.vector.tensor_mul(OT[z][0:64, ls * 512:(ls + 1) * 512],
                                         OT[z][0:64, ls * 512:(ls + 1) * 512],
                                         rr[:])

            if dbg:
                nc.sync.dma_start(dbg_t["dot"].ap(), OT[0][:])

            # ---------------- phase 4: pack z-pairs [128, L] ----------------
            PK = [vtpk.tile([128, L], BF16, tag="vtpk", name=f"PK{i}") for i in range(2)]
            for zp in range(2):
                nc.sync.dma_start(PK[zp][0:64, :], OT[2 * zp][0:64, :])
                nc.sync.dma_start(PK[zp][64:128, :], OT[2 * zp + 1][0:64, :])

            if dbg:
                nc.sync.dma_start(dbg_t["dpk"].ap(), PK[0][:])

            # ---------------- phase 5: out_proj partial ----------------
            # ReduceScatter is chunked along m (4 x 256 rows) so comm for
            # chunk j overlaps compute of chunk j+1.
            partial = dram.tile([HID, L], BF16)
            rs_chunks = []
            for j in range(4):
                rs_chunks.append(dram.tile([64, L], BF16, name=f"rsch{j}",
                                           tag=f"rsch{j}"))
            for j in range(4):
                for mc in (2 * j, 2 * j + 1):
                    for lt in range(8):
                        ps = ps_s.tile([128, 512], F32, tag="s")
                        nc.tensor.matmul(ps[:],
                                         wo_sb[0][:, mc * 128:(mc + 1) * 128],
                                         PK[0][:, lt * 512:(lt + 1) * 512],
                                         start=True, stop=False)
                        nc.tensor.matmul(ps[:],
                                         wo_sb[1][:, mc * 128:(mc + 1) * 128],
                                         PK[1][:, lt * 512:(lt + 1) * 512],
                                         start=False, stop=True)
                        st = stg.tile([128, 512], BF16, tag="st")
                        nc.vector.tensor_scalar_add(st[:], ps[:],
                                                    ob_sb[:, mc:mc + 1])
                        nc.sync.dma_start(
                            partial[mc * 128:(mc + 1) * 128,
                                    lt * 512:(lt + 1) * 512],
                            st[:])
                nc.gpsimd.collective_compute(
                    "ReduceScatter", mybir.AluOpType.add,
                    replica_groups=GROUPS,
                    ins=[partial[2 * j * 128:(2 * j + 2) * 128, :]],
                    outs=[rs_chunks[j].opt()])

            if dbg:
                nc.sync.dma_start(dbg_t["dpp"].ap(), partial[0:128, :])

            # ---------------- phase 6: emit rs chunks as f32 ----------------
            for j in range(4):
                for lt in range(4):
                    bt = stg.tile([64, 1024], BF16, tag="fin_b")
                    nc.sync.dma_start(
                        bt[:],
                        rs_chunks[j][:, lt * 1024:(lt + 1) * 1024])
                    ft = stg.tile([64, 1024], F32, tag="fin_f")
                    nc.vector.tensor_copy(ft[:], bt[:])
                    nc.sync.dma_start(
                        out[j * 64:(j + 1) * 64, lt * 1024:(lt + 1) * 1024],
                        ft[:])

    nc.compile()
    return nc


_CACHE = {}


def _get_nc(rand_idx_key, rand_idx):
    if rand_idx_key not in _CACHE:
        _CACHE[rand_idx_key] = _build(rand_idx)
    return _CACHE[rand_idx_key]


def make_in_maps(query, q_proj, q_bias, k_proj, k_bias, v_proj, v_bias,
                 out_proj, out_bias, rand_indices, h, block_size):
    bf = ml_dtypes.bfloat16
    scale = DH ** -0.5
    in_maps = []
    for c in range(NCORES):
        n, g = c // 4, c % 4
        cols = slice(g * 256, (g + 1) * 256)
        m = {
            "xt": np.ascontiguousarray(query[:, n, :].T).astype(bf),
            "wqT": np.ascontiguousarray((q_proj[cols, :] * scale).T).astype(bf),
            "wkT": np.ascontiguousarray(k_proj[cols, :].T).astype(bf),
            "wvT": np.ascontiguousarray(v_proj[cols, :].T).astype(bf),
            "woT": np.ascontiguousarray(out_proj[:, cols].T).astype(bf),
            "qb": np.ascontiguousarray(q_bias[cols] * scale).astype(np.float32),
            "kb": np.ascontiguousarray(k_bias[cols]).astype(np.float32),
            "vb": np.ascontiguousarray(v_bias[cols]).astype(np.float32),
            "ob": np.ascontiguousarray(out_bias * 0.25).astype(np.float32),
        }
        in_maps.append(m)
    return in_maps


def kernel(query, q_proj, q_bias, k_proj, k_bias, v_proj, v_bias,
           out_proj, out_bias, rand_indices, h, block_size, _results_hook=None):
    query = np.asarray(query, dtype=np.float32)
    q_proj = np.asarray(q_proj, dtype=np.float32)
    q_bias = np.asarray(q_bias, dtype=np.float32)
    k_proj = np.asarray(k_proj, dtype=np.float32)
    k_bias = np.asarray(k_bias, dtype=np.float32)
    v_proj = np.asarray(v_proj, dtype=np.float32)
    v_bias = np.asarray(v_bias, dtype=np.float32)
    out_proj = np.asarray(out_proj, dtype=np.float32)
    out_bias = np.asarray(out_bias, dtype=np.float32)
    rand_indices = np.asarray(rand_indices)
    assert int(h) == HEADS and int(block_size) == B
    assert query.shape == (L, 2, HID)
    assert rand_indices.shape == (NB - 2, RAND)
    rand_idx = [(int(r0), int(r1)) for r0, r1 in rand_indices]
    assert all(2 <= r < NB for p in rand_idx for r in p)

    nc = _get_nc(tuple(map(tuple, rand_idx)), rand_idx)
    in_maps = make_in_maps(query, q_proj, q_bias, k_proj, k_bias, v_proj,
                           v_bias, out_proj, out_bias, rand_indices, h,
                           block_size)
    res = run_bass_kernel_spmd(nc, in_maps, core_ids=list(range(NCORES)))
    if _results_hook is not None:
        _results_hook(res)

    return assemble([res.results[c]["out"] for c in range(NCORES)])


def assemble(outs):
    """outs[c]: [256, L] f32; chunk j rows j*64..(j+1)*64 hold global
    m rows 256*j + 64*g .. +64 for group rank g = c%4."""
    output = np.empty((L, 2, HID), dtype=np.float32)
    for c in range(NCORES):
        n, g = c // 4, c % 4
        for j in range(4):
            output[:, n, 256 * j + 64 * g:256 * j + 64 * g + 64] = \
                outs[c][j * 64:(j + 1) * 64].T
    return output
